# revision 37
# baseline (speedup 1.0000x reference)
"""Trainium2 Bass kernel for DCTEncoderLayer.

Computes, for rgb_images_batch [32, 3, 512, 512] f32:
  ycbcr' = 2*rgb_to_ycbcr(rgb) - 1                 (per-pixel 3x3 channel mix, affine)
  32x32 block DCT per channel, coefficients scaled by (2/32)*c_u*c_v,
  output [32, 3*1024, 16, 16] with the frequency axis sorted by |(v,u)|.

Strategy (pure data parallel over batch, 4 images per NeuronCore):
  The 2D DCT is separable: coeff = Cs @ block @ Cs.T with Cs[v,y] =
  cos((2y+1)v*pi/64) * c_v / 4.  The YCbCr channel mix is linear and is
  folded into the stage-1 weights (contraction runs over (channel, y));
  feeding the device rgb-0.5 makes the affine offset exact.

  Key trick: stage 1 runs with the IMAGE chunk as the matmul's stationary
  operand and the 96x96 mixed DCT weight as the moving operand, so the
  PSUM result arrives TRANSPOSED, [x (128 partitions), (c,v)] — no DVE
  stream-transpose is ever needed, and all later passes run on 128
  partitions instead of 96 (25% fewer columns).
  Per mega-tile (2 block-rows of one image = 8 chunks of 128 x-columns):
    stage1 (f16, PE):  8x  t1T[x128, (c,v)96] = img_chunk.T @ W1m
                       accumulated side by side in PSUM [128, 768]
    stage-cast (ACT):  PSUM f32 -> SBUF f16                  [128, 768]
    stage2 (f16, PE):  2x  o2[(gxl,u)128, (k,c,v)384] = W2p.T @ t1s
                       with W2p = blockdiag(Cs.T x4), constant
    out-cast (ACT/DVE alternating, deferred one mega-tile):  [128, 768]
    f16 DMA out, 2 mega-tiles per transfer.
  The host reassembles/permutes axes and applies the frequency sort.
"""

import os
import sys

try:
    import concourse.bass  # noqa: F401
except ImportError:  # bare interpreter without the axon site paths
    sys.path.insert(0, "/opt/trn_rl_repo")

import numpy as np

import concourse.bacc as bacc
import concourse.bass as bass
import concourse.mybir as mybir
import concourse.tile as tile
from concourse.bass_utils import run_bass_kernel_spmd

F32 = mybir.dt.float32
F16 = mybir.dt.float16

BS = 32            # DCT block size
N_CORES = 8
B_PER_CORE = 4     # batch images per core
NH = 16            # blocks per row/column (512/32)
MEGAS = B_PER_CORE * NH // 2   # 32 mega-tiles of 2 block-rows each
GRP_IN = 4         # mega-tiles per input DMA  (8 DMAs)
GRP_OUT = 2        # mega-tiles per output DMA (16 DMAs)

_STATE = {}
LAST_RESULT = None  # BassKernelResults of the most recent run (for profiling)


def _dct_mat():
    """Cs[v, y] = cos((2y+1) v pi / 64) * c_v / 4  (f64)."""
    y = np.arange(BS)
    v = np.arange(BS)[:, None]
    c = np.cos((2 * y + 1) * v * np.pi / (2 * BS))
    c[0, :] *= 1.0 / np.sqrt(2.0)
    return c / 4.0


def _sort_idx():
    # must replicate the reference's argsort (default kind) exactly,
    # including its tie order for equal |(v,u)|
    mag = np.zeros((BS, BS), dtype=np.float64)
    for v in range(BS):
        for u in range(BS):
            mag[v, u] = np.linalg.norm(np.array([v, u], dtype=np.int64))
    return np.argsort(mag.reshape(-1))


def _constants():
    cs = _dct_mat()
    # rows (y', cb', cr') of the linear part of 2*rgb_to_ycbcr(rgb)-1, in (r,g,b)
    a2 = np.array(
        [
            [2 * 0.299, 2 * 0.587, 2 * 0.114],
            [2 * 0.564 * -0.299, 2 * 0.564 * -0.587, 2 * 0.564 * (1 - 0.114)],
            [2 * 0.713 * (1 - 0.299), 2 * 0.713 * -0.587, 2 * 0.713 * -0.114],
        ],
        np.float64,
    )
    w1 = np.zeros((96, 96))  # [(c', y), (c, v)]
    for cp in range(3):
        for c in range(3):
            w1[cp * 32 : (cp + 1) * 32, c * 32 : (c + 1) * 32] = a2[c, cp] * cs.T
    # stage-2 stationary: [(gxl, x'), (gxl, u)] block diagonal over gxl (4x)
    w2 = np.zeros((128, 128))
    for gxl in range(4):
        w2[gxl * 32 : (gxl + 1) * 32, gxl * 32 : (gxl + 1) * 32] = cs.T
    return w1.astype(np.float16), w2.astype(np.float16)


def _build_program():
    nc = bacc.Bacc(trn_type="TRN2")
    # per mega-tile: [96=(c,y_local), 1024=(2 block-rows x 512 x)]
    x = nc.dram_tensor("x", [MEGAS // GRP_IN, 96, GRP_IN * 1024], F16,
                       kind="ExternalInput")
    w1 = nc.dram_tensor("w1", [96, 96], F16, kind="ExternalInput")
    w2 = nc.dram_tensor("w2", [128, 128], F16, kind="ExternalInput")
    out = nc.dram_tensor("out", [MEGAS // GRP_OUT, 128, GRP_OUT * 768], F16,
                         kind="ExternalOutput")

    with tile.TileContext(nc) as tc:
        with (
            tc.tile_pool(name="const", bufs=1) as constp,
            tc.tile_pool(name="xin", bufs=6) as xinp,
            tc.tile_pool(name="sb", bufs=10) as sb,
            tc.tile_pool(name="psA", bufs=2, space="PSUM") as psA,
            tc.tile_pool(name="psB", bufs=2, space="PSUM") as psB,
        ):
            w1s = constp.tile([96, 96], F16)
            w2s = constp.tile([128, 128], F16)
            nc.scalar.dma_start(w1s[:], w1[:])
            nc.scalar.dma_start(w2s[:], w2[:])

            osbs = {}
            o2ps = {}

            # out-casts run one mega-tile late so they never head-of-line
            # block the next mega-tile's stage work on the same engine
            def emit_outcast(m):
                g, j = m // GRP_OUT, m % GRP_OUT
                if j == 0:
                    osbs[g] = sb.tile([128, GRP_OUT * 768], F16,
                                      name="osb", tag="osb")
                oslice = osbs[g][:, j * 768 : (j + 1) * 768]
                o2p = o2ps.pop(m)
                if m % 8 == 0:
                    nc.scalar.copy(oslice, o2p[:])
                else:
                    nc.vector.tensor_copy(oslice, o2p[:])
                if j == GRP_OUT - 1:
                    nc.gpsimd.dma_start(out[g], osbs.pop(g)[:])

            # input prefetch: issue group g's DMA two groups ahead of use,
            # split into per-mega transfers for the first group so the very
            # first matmul starts after ~0.6us instead of ~2.5us
            xins = {}

            def emit_indma(g):
                if g == 0:
                    # separate per-mega tiles: dependency tracking is
                    # tile-granular, so mega 0's first matmul must not wait
                    # on the whole group's transfers
                    xins[g] = [
                        xinp.tile([96, 1024], F16, name="xin0", tag=f"xin0_{jj}")
                        for jj in range(GRP_IN)
                    ]
                    for jj in range(GRP_IN):
                        nc.sync.dma_start(
                            xins[g][jj][:],
                            x[g, :, jj * 1024 : (jj + 1) * 1024],
                        )
                else:
                    xins[g] = xinp.tile([96, GRP_IN * 1024], F16,
                                        name="xin", tag="xin")
                    nc.sync.dma_start(xins[g][:], x[g])

            emit_indma(0)
            emit_indma(1)

            for m in range(MEGAS):
                g, j = m // GRP_IN, m % GRP_IN
                if j == 0 and g + 2 <= MEGAS // GRP_IN - 1:
                    emit_indma(g + 2)
                if g == 0:
                    img = xins[g][j][:]
                else:
                    img = xins[g][:, j * 1024 : (j + 1) * 1024]
                # stage 1: img chunk stationary -> transposed PSUM output.
                # chunk k writes [k*128, k*128+96) so no matmul output
                # crosses a 2KB PSUM bank boundary (96-wide packing would)
                t1p = psA.tile([128, 1024], F32, tag="t1p")
                for k in range(8):
                    nc.tensor.matmul(
                        t1p[:, k * 128 : k * 128 + 96],
                        img[:, k * 128 : (k + 1) * 128],
                        w1s[:],
                        start=True,
                        stop=True,
                    )
                # stage cast on ACT: PSUM f32 -> SBUF f16, packing 8x96 out
                # of the 8x128 padded layout via strided APs
                t1s = sb.tile([128, 768], F16, tag="t1s")
                nc.scalar.copy(
                    t1s[:].rearrange("p (k j) -> p k j", j=96),
                    t1p[:].rearrange("p (k w) -> p k w", w=128)[:, :, 0:96],
                )
                # stage 2: constant block-diag DCT stationary; split 512+256
                # to keep each output inside one PSUM bank
                o2p = psB.tile([128, 768], F32, tag="o2p")
                for lo, hi in ((0, 512), (512, 768)):
                    nc.tensor.matmul(
                        o2p[:, lo:hi],
                        w2s[:],
                        t1s[:, lo:hi],
                        start=True,
                        stop=True,
                    )
                o2ps[m] = o2p
                if m > 0:
                    emit_outcast(m - 1)
            emit_outcast(MEGAS - 1)

    nc.finalize()
    return nc


def _get_program():
    if "nc" not in _STATE:
        _STATE["nc"] = _build_program()
        _STATE["consts"] = _constants()
        _STATE["sort_idx"] = _sort_idx()
    return _STATE["nc"]


def kernel(**inputs):
    global LAST_RESULT
    rgb = np.asarray(inputs["rgb_images_batch"], np.float32)
    assert rgb.shape == (N_CORES * B_PER_CORE, 3, 512, 512)
    # centering makes the YCbCr affine offset vanish (row sums of the cb/cr
    # mix are 0 and the y row sums to 2 -> offset 2*0.5-1=0 for every channel)
    B = N_CORES * B_PER_CORE
    xs = rgb.reshape(B, 3, NH, 32, 512).transpose(0, 2, 1, 3, 4)
    xs = (np.ascontiguousarray(xs).reshape(B, NH, 96, 512)
          - np.float32(0.5)).astype(np.float16)
    # mega-tile layout: [(b, gy-pair), 96, (gy-parity, x)]
    xt = xs.reshape(B, NH // 2, 2, 96, 512).transpose(0, 1, 3, 2, 4)
    xt = np.ascontiguousarray(xt).reshape(B, NH // 2, 96, 1024)
    nc = _get_program()
    w1, w2 = _STATE["consts"]
    sort_idx = _STATE["sort_idx"]

    in_maps = []
    for c in range(N_CORES):
        xc = xt[c * B_PER_CORE : (c + 1) * B_PER_CORE].reshape(MEGAS, 96, 1024)
        xg = xc.reshape(MEGAS // GRP_IN, GRP_IN, 96, 1024).transpose(0, 2, 1, 3)
        xg = np.ascontiguousarray(xg).reshape(MEGAS // GRP_IN, 96, GRP_IN * 1024)
        in_maps.append({"x": xg, "w1": w1, "w2": w2})
    trace = os.environ.get("KERNEL_TRACE", "0") == "1"
    res = run_bass_kernel_spmd(
        nc, in_maps, core_ids=list(range(N_CORES)), trace=trace
    )
    LAST_RESULT = res

    outs = []
    for c in range(N_CORES):
        dev = np.asarray(res.results[c]["out"], np.float32)  # [16, 128, 1536]
        # free layout per group: (mj 2 megas, ru 2 row-units, k 4, c 3, v 32)
        # partition: (gxl 4, u 32); row-unit index = (grp, mj, ru) = (b, gy)
        a = dev.reshape(MEGAS // GRP_OUT, 4, 32, GRP_OUT, 2, 4, 3, 32)
        a = a.transpose(0, 3, 4, 6, 7, 2, 5, 1)  # grp, mj, ru, c, v, u, k, gxl
        a = np.ascontiguousarray(a).reshape(B_PER_CORE, NH, 3, 32, 32, 16)
        a = a.transpose(0, 2, 3, 4, 1, 5)        # b, c, v, u, gy, gx
        a = np.ascontiguousarray(a).reshape(B_PER_CORE, 3, 1024, NH, NH)
        a = a[:, :, sort_idx, :, :]
        outs.append(a.reshape(B_PER_CORE, 3 * 1024, NH, NH))
    return np.concatenate(outs, axis=0)


# revision 38
# speedup vs baseline: 1.1871x; 1.1871x over previous
"""Trainium2 Bass kernel for DCTEncoderLayer.

Computes, for rgb_images_batch [32, 3, 512, 512] f32:
  ycbcr' = 2*rgb_to_ycbcr(rgb) - 1                 (per-pixel 3x3 channel mix, affine)
  32x32 block DCT per channel, coefficients scaled by (2/32)*c_u*c_v,
  output [32, 3*1024, 16, 16] with the frequency axis sorted by |(v,u)|.

Strategy (pure data parallel over batch, 4 images per NeuronCore):
  The 2D DCT is separable: coeff = Cs @ block @ Cs.T with Cs[v,y] =
  cos((2y+1)v*pi/64) * c_v / 4.  The YCbCr channel mix is linear and is
  folded into the stage-1 weights (contraction runs over (channel, y));
  feeding the device rgb-0.5 makes the affine offset exact.

  Key trick: stage 1 runs with the IMAGE chunk as the matmul's stationary
  operand and the 96x96 mixed DCT weight as the moving operand, so the
  PSUM result arrives TRANSPOSED, [x (128 partitions), (c,v)] — no DVE
  stream-transpose is ever needed, and all later passes run on 128
  partitions instead of 96 (25% fewer columns).
  Per mega-tile (2 block-rows of one image = 8 chunks of 128 x-columns):
    stage1 (f16, PE):  8x  t1T[x128, (c,v)96] = img_chunk.T @ W1m
                       accumulated side by side in PSUM [128, 768]
    stage-cast (ACT):  PSUM f32 -> SBUF f16                  [128, 768]
    stage2 (f16, PE):  2x  o2[(gxl,u)128, (k,c,v)384] = W2p.T @ t1s
                       with W2p = blockdiag(Cs.T x4), constant
    out-cast (ACT/DVE alternating, deferred one mega-tile):  [128, 768]
    f16 DMA out, 2 mega-tiles per transfer.
  The host reassembles/permutes axes and applies the frequency sort.
"""

import os
import sys

try:
    import concourse.bass  # noqa: F401
except ImportError:  # bare interpreter without the axon site paths
    sys.path.insert(0, "/opt/trn_rl_repo")

import numpy as np

import concourse.bacc as bacc
import concourse.bass as bass
import concourse.mybir as mybir
import concourse.tile as tile
from concourse.bass_utils import run_bass_kernel_spmd

F32 = mybir.dt.float32
F16 = mybir.dt.float16

BS = 32            # DCT block size
N_CORES = 8
B_PER_CORE = 4     # batch images per core
NH = 16            # blocks per row/column (512/32)
MEGAS = B_PER_CORE * NH // 2   # 32 mega-tiles of 2 block-rows each
GRP_IN = 4         # mega-tiles per input DMA  (8 DMAs)
GRP_OUT = 2        # mega-tiles per output DMA (16 DMAs)

_STATE = {}
LAST_RESULT = None  # BassKernelResults of the most recent run (for profiling)


def _dct_mat():
    """Cs[v, y] = cos((2y+1) v pi / 64) * c_v / 4  (f64)."""
    y = np.arange(BS)
    v = np.arange(BS)[:, None]
    c = np.cos((2 * y + 1) * v * np.pi / (2 * BS))
    c[0, :] *= 1.0 / np.sqrt(2.0)
    return c / 4.0


def _sort_idx():
    # must replicate the reference's argsort (default kind) exactly,
    # including its tie order for equal |(v,u)|
    mag = np.zeros((BS, BS), dtype=np.float64)
    for v in range(BS):
        for u in range(BS):
            mag[v, u] = np.linalg.norm(np.array([v, u], dtype=np.int64))
    return np.argsort(mag.reshape(-1))


def _constants():
    cs = _dct_mat()
    # rows (y', cb', cr') of the linear part of 2*rgb_to_ycbcr(rgb)-1, in (r,g,b)
    a2 = np.array(
        [
            [2 * 0.299, 2 * 0.587, 2 * 0.114],
            [2 * 0.564 * -0.299, 2 * 0.564 * -0.587, 2 * 0.564 * (1 - 0.114)],
            [2 * 0.713 * (1 - 0.299), 2 * 0.713 * -0.587, 2 * 0.713 * -0.114],
        ],
        np.float64,
    )
    w1 = np.zeros((96, 96))  # [(c', y), (c, v)]
    for cp in range(3):
        for c in range(3):
            w1[cp * 32 : (cp + 1) * 32, c * 32 : (c + 1) * 32] = a2[c, cp] * cs.T
    # stage-2 stationary: [(gxl, x'), (gxl, u)] block diagonal over gxl (4x)
    w2 = np.zeros((128, 128))
    for gxl in range(4):
        w2[gxl * 32 : (gxl + 1) * 32, gxl * 32 : (gxl + 1) * 32] = cs.T
    return w1.astype(np.float16), w2.astype(np.float16)


def _build_program():
    nc = bacc.Bacc(trn_type="TRN2")
    # per mega-tile: [96=(c,y_local), 1024=(2 block-rows x 512 x)]
    x = nc.dram_tensor("x", [MEGAS // GRP_IN, 96, GRP_IN * 1024], F16,
                       kind="ExternalInput")
    w1 = nc.dram_tensor("w1", [96, 96], F16, kind="ExternalInput")
    w2 = nc.dram_tensor("w2", [128, 128], F16, kind="ExternalInput")
    out = nc.dram_tensor("out", [MEGAS // GRP_OUT, 128, GRP_OUT * 768], F16,
                         kind="ExternalOutput")

    with tile.TileContext(nc) as tc:
        with (
            tc.tile_pool(name="const", bufs=1) as constp,
            tc.tile_pool(name="xin", bufs=6) as xinp,
            tc.tile_pool(name="sb", bufs=10) as sb,
            tc.tile_pool(name="psA", bufs=2, space="PSUM") as psA,
            tc.tile_pool(name="psB", bufs=2, space="PSUM") as psB,
        ):
            w1s = constp.tile([96, 96], F16)
            w2s = constp.tile([128, 128], F16)
            nc.scalar.dma_start(w1s[:], w1[:])
            nc.scalar.dma_start(w2s[:], w2[:])

            osbs = {}
            o2ps = {}

            # out-casts run one mega-tile late so they never head-of-line
            # block the next mega-tile's stage work on the same engine
            def emit_outcast(m):
                g, j = m // GRP_OUT, m % GRP_OUT
                if j == 0:
                    osbs[g] = sb.tile([128, GRP_OUT * 768], F16,
                                      name="osb", tag="osb")
                oslice = osbs[g][:, j * 768 : (j + 1) * 768]
                o2p = o2ps.pop(m)
                if m % 8 == 0:
                    nc.scalar.copy(oslice, o2p[:])
                else:
                    nc.vector.tensor_copy(oslice, o2p[:])
                if j == GRP_OUT - 1:
                    nc.gpsimd.dma_start(out[g], osbs.pop(g)[:])

            # input prefetch: issue group g's DMA two groups ahead of use,
            # split into per-mega transfers for the first group so the very
            # first matmul starts after ~0.6us instead of ~2.5us
            xins = {}

            def emit_indma(g):
                if g == 0:
                    # separate per-mega tiles: dependency tracking is
                    # tile-granular, so mega 0's first matmul must not wait
                    # on the whole group's transfers
                    xins[g] = [
                        xinp.tile([96, 1024], F16, name="xin0", tag=f"xin0_{jj}")
                        for jj in range(GRP_IN)
                    ]
                    for jj in range(GRP_IN):
                        nc.sync.dma_start(
                            xins[g][jj][:],
                            x[g, :, jj * 1024 : (jj + 1) * 1024],
                        )
                else:
                    xins[g] = xinp.tile([96, GRP_IN * 1024], F16,
                                        name="xin", tag="xin")
                    nc.sync.dma_start(xins[g][:], x[g])

            emit_indma(0)
            emit_indma(1)

            for m in range(MEGAS):
                g, j = m // GRP_IN, m % GRP_IN
                if j == 0 and g + 2 <= MEGAS // GRP_IN - 1:
                    emit_indma(g + 2)
                if g == 0:
                    img = xins[g][j][:]
                else:
                    img = xins[g][:, j * 1024 : (j + 1) * 1024]
                # stage 1: img chunk stationary -> transposed PSUM output.
                # chunk k writes [k*128, k*128+96) so no matmul output
                # crosses a 2KB PSUM bank boundary (96-wide packing would)
                t1p = psA.tile([128, 1024], F32, tag="t1p")
                for k in range(8):
                    nc.tensor.matmul(
                        t1p[:, k * 128 : k * 128 + 96],
                        img[:, k * 128 : (k + 1) * 128],
                        w1s[:],
                        start=True,
                        stop=True,
                    )
                # stage cast on ACT: PSUM f32 -> SBUF f16, packing 8x96 out
                # of the 8x128 padded layout via strided APs
                t1s = sb.tile([128, 768], F16, tag="t1s")
                nc.scalar.copy(
                    t1s[:].rearrange("p (k j) -> p k j", j=96),
                    t1p[:].rearrange("p (k w) -> p k w", w=128)[:, :, 0:96],
                )
                # stage 2: constant block-diag DCT stationary; split 512+256
                # to keep each output inside one PSUM bank
                o2p = psB.tile([128, 768], F32, tag="o2p")
                for lo, hi in ((0, 512), (512, 768)):
                    nc.tensor.matmul(
                        o2p[:, lo:hi],
                        w2s[:],
                        t1s[:, lo:hi],
                        start=True,
                        stop=True,
                    )
                o2ps[m] = o2p
                emit_outcast(m)

    nc.finalize()
    return nc


def _get_program():
    if "nc" not in _STATE:
        _STATE["nc"] = _build_program()
        _STATE["consts"] = _constants()
        _STATE["sort_idx"] = _sort_idx()
    return _STATE["nc"]


def kernel(**inputs):
    global LAST_RESULT
    rgb = np.asarray(inputs["rgb_images_batch"], np.float32)
    assert rgb.shape == (N_CORES * B_PER_CORE, 3, 512, 512)
    # centering makes the YCbCr affine offset vanish (row sums of the cb/cr
    # mix are 0 and the y row sums to 2 -> offset 2*0.5-1=0 for every channel)
    B = N_CORES * B_PER_CORE
    xs = rgb.reshape(B, 3, NH, 32, 512).transpose(0, 2, 1, 3, 4)
    xs = (np.ascontiguousarray(xs).reshape(B, NH, 96, 512)
          - np.float32(0.5)).astype(np.float16)
    # mega-tile layout: [(b, gy-pair), 96, (gy-parity, x)]
    xt = xs.reshape(B, NH // 2, 2, 96, 512).transpose(0, 1, 3, 2, 4)
    xt = np.ascontiguousarray(xt).reshape(B, NH // 2, 96, 1024)
    nc = _get_program()
    w1, w2 = _STATE["consts"]
    sort_idx = _STATE["sort_idx"]

    in_maps = []
    for c in range(N_CORES):
        xc = xt[c * B_PER_CORE : (c + 1) * B_PER_CORE].reshape(MEGAS, 96, 1024)
        xg = xc.reshape(MEGAS // GRP_IN, GRP_IN, 96, 1024).transpose(0, 2, 1, 3)
        xg = np.ascontiguousarray(xg).reshape(MEGAS // GRP_IN, 96, GRP_IN * 1024)
        in_maps.append({"x": xg, "w1": w1, "w2": w2})
    trace = os.environ.get("KERNEL_TRACE", "0") == "1"
    res = run_bass_kernel_spmd(
        nc, in_maps, core_ids=list(range(N_CORES)), trace=trace
    )
    LAST_RESULT = res

    outs = []
    for c in range(N_CORES):
        dev = np.asarray(res.results[c]["out"], np.float32)  # [16, 128, 1536]
        # free layout per group: (mj 2 megas, ru 2 row-units, k 4, c 3, v 32)
        # partition: (gxl 4, u 32); row-unit index = (grp, mj, ru) = (b, gy)
        a = dev.reshape(MEGAS // GRP_OUT, 4, 32, GRP_OUT, 2, 4, 3, 32)
        a = a.transpose(0, 3, 4, 6, 7, 2, 5, 1)  # grp, mj, ru, c, v, u, k, gxl
        a = np.ascontiguousarray(a).reshape(B_PER_CORE, NH, 3, 32, 32, 16)
        a = a.transpose(0, 2, 3, 4, 1, 5)        # b, c, v, u, gy, gx
        a = np.ascontiguousarray(a).reshape(B_PER_CORE, 3, 1024, NH, NH)
        a = a[:, :, sort_idx, :, :]
        outs.append(a.reshape(B_PER_CORE, 3 * 1024, NH, NH))
    return np.concatenate(outs, axis=0)
